# revision 76
# baseline (speedup 1.0000x reference)
"""DDNLoss (depth-distribution focal loss) Trainium2 kernel, 8-core data-parallel.

Strategy (per core = one image of the batch):
  * depth_logits [81, 30720] loaded in 6 contiguous chunks (issued at t=0),
    ACT exp -> bf16, then 16 one-hot-column matmuls per chunk partition-reduce
    the 81 channels into a [16, 320] PSUM tile whose row i is image row
    16c+i's softmax denominator; one DVE copy evicts it straight into the
    pixel-major s_b [96, 320].
  * The 17 candidate channels (16 sorted box bins + background 80) are
    gathered with 2 indirect DMAs (f32 -> bf16 cast) and restacked to
    lstack [96(v), 17(k), 320(u)] with 17 partition-expand DMAs spread
    across the pool/vector/sync rings.
  * Rasterization (min-depth box wins) is an arithmetic min-encode:
    enc = lambda + 32k + 16 + pen, pen built by ONE K=18 bf16 matmul per
    512-col block: rows 0..16 carry 2048*(2-rowmask[k,v]) through a
    block-diagonal expansion, row 17 broadcasts the column-mask encode
    32k+16-2048*colmask (all values bf16-exact). A strided tensor_reduce
    min over k yields m* = 32k* + 16 + lambda* per pixel.
  * Focal loss is elementwise in pixel-major layout; per-partition row sums
    are returned and the host adds the 8 per-core partials.
"""

import sys

sys.path.insert(0, "/opt/trn_rl_repo")

import numpy as np

B, C, H, W = 8, 81, 96, 320
F = H * W
NBOX, NCAND = 16, 17  # 16 boxes + background
ALPHA = 0.25
FG_W, BG_W = 13.0, 1.0
DEPTH_MIN, DEPTH_MAX, NUM_BINS = 0.001, 60.0, 80

STRIDE = 32.0  # rank stride in the min-encode
OFF = 16.0  # lambda offset so the payload is positive
BIG = 2048.0  # uncovered-box penalty (bf16-exact composites)
UBLK = 80  # u-block size for the pen/enc/reduce pipeline (4 blocks)
# uneven exp/S chunks: small first chunks prime the pipeline, small last
# chunks shorten the tail; groups of 32 rows share one PSUM tile so the
# eviction lands on 0/32/64 partition boundaries
CHUNK_ROWS = (8, 24, 24, 8, 24, 8)
NCHUNK = len(CHUNK_ROWS)
MAXROWS = max(CHUNK_ROWS)


def _groups(chunk_rows):
    group_of, group_last, r = [], [], 0
    for c, n in enumerate(chunk_rows):
        g = r // 32
        group_of.append(g)
        r += n
        assert r <= (g + 1) * 32, "chunk straddles a 32-row psum group"
        if r % 32 == 0:
            group_last.append(c)
    assert r == 96 and len(group_last) == 3
    return tuple(group_of), tuple(group_last)


GROUP_OF, GROUP_LAST = _groups(CHUNK_ROWS)

_PROG = None  # cached program


def _build_program():
    from concourse import bass, bacc, tile, mybir

    f32 = mybir.dt.float32
    bf16 = mybir.dt.bfloat16
    i32 = mybir.dt.int32
    AF = mybir.ActivationFunctionType
    OP = mybir.AluOpType

    nc = bacc.Bacc(
        "TRN2",
        target_bir_lowering=False,
        debug=False,
        enable_asserts=False,
        dynamic_dma_scratch_size=65536,
    )

    # ---- DRAM I/O (per-core) ----
    # bf16 copy of the logits (host-cast): the S path only needs exp() of
    # them and 2^-9 relative error is far inside the loss tolerance, so the
    # kernel streams half the bytes. The f32 original is never read.
    Lb = nc.dram_tensor("logitsb", [C, F], bf16, kind="ExternalInput")
    lstack_d = nc.dram_tensor("lstack", [H, NCAND * W], bf16, kind="ExternalInput")
    wk_d = nc.dram_tensor("wk", [NCAND + 1, H], bf16, kind="ExternalInput")
    bd_d = nc.dram_tensor("bd", [NCAND + 1, 4 * UBLK * NCAND], bf16, kind="ExternalInput")
    slider_d = nc.dram_tensor("slider", [C, 65], bf16, kind="ExternalInput")
    out_d = nc.dram_tensor("out", [H, 1], f32, kind="ExternalOutput")
    import os

    dbg = os.environ.get("KERNEL_DEBUG") == "1"
    if dbg:
        dbg_m = nc.dram_tensor("dbg_m", [H, W], f32, kind="ExternalOutput")
        dbg_s = nc.dram_tensor("dbg_s", [H, W], f32, kind="ExternalOutput")

    PENW = UBLK * NCAND  # 1360 pen columns per quarter

    with tile.TileContext(nc) as tc:
        with (
            tc.tile_pool(name="persist", bufs=1) as pp,
            tc.tile_pool(name="lchunk", bufs=6) as cp,
            tc.tile_pool(name="echunk", bufs=2) as xp,
            tc.tile_pool(name="enc", bufs=1) as ep,
            tc.tile_pool(name="spsum", bufs=2, space="PSUM") as sp,
            tc.tile_pool(name="ppsum", bufs=6, space="PSUM") as qp,
        ):
            # ---------- constant / small input loads first, then chunk 0,
            # then the host-stacked candidate tensor (1MB, gates the enc
            # chain), then the remaining logits chunks. All plain sync-ring
            # HWDGE loads: no SWDGE, no gpsimd, no drains.
            row0 = np.cumsum((0,) + CHUNK_ROWS)
            slider = pp.tile([C, 65], bf16)
            wk = pp.tile([NCAND + 1, H], bf16)
            bd = pp.tile([NCAND + 1, 4 * PENW], bf16)

            def load_consts():
                nc.sync.dma_start(slider[:], slider_d[:])
                nc.sync.dma_start(wk[:], wk_d[:])
                nc.sync.dma_start(bd[:], bd_d[:])

            # Loads alternate HWDGE (sync, queues 0-8) and SWDGE (gpsimd,
            # queues 9-15) rings so all 16 DMA queues stream. Sync-ring
            # completions are tracked per-DMA, so those all issue upfront;
            # SWDGE completions are tracked by a coarse FIFO tick (any
            # compute emitted after a pool DMA waits for ALL earlier pool
            # DMAs), so the pool-ring dma_starts are interleaved into the
            # compute emission just before their consumers.
            lcs = {}

            def load_chunk(c_):
                # two half-partition DMAs per chunk: halves the per-chunk
                # completion latency (descs per call halve)
                lc = cp.tile([C, MAXROWS * W], bf16, tag="lc")
                ring = nc.sync if c_ % 2 == 0 else nc.gpsimd
                cw = CHUNK_ROWS[c_] * W
                for p0, p1 in ((0, 41), (41, C)):
                    ring.dma_start(
                        lc[p0:p1, :cw],
                        Lb[p0:p1, row0[c_] * W : row0[c_ + 1] * W],
                    )
                lcs[c_] = lc

            load_chunk(0)
            load_consts()
            load_chunk(2)
            load_chunk(4)
            lstack = pp.tile([H, NCAND, W], bf16)

            # ---------- pen matmuls (bf16, K=18) ----------
            # pen[v, u*17+k] = 2048*(2-rowm[k,v]) + 32k+16 - 2048*colm[k,u]
            # Twelve 1-PSUM-bank blocks of 28/28/24 u-groups per quarter so
            # pen matmuls never couple the PE queue to the (gather-gated)
            # enc chain: with bufs=6, block b+6 only recycles a bank whose
            # enc read finished long before.
            pens = []
            s_b = pp.tile([H, W], f32)  # softmax denominator, pixel-major
            mstar = pp.tile([H, W], f32)
            UG = (28, 28, 24)  # u-groups per block (x17 cols)

            def pen_blk(b):
                q, j = b // 3, b % 3
                ncol = UG[j] * NCAND
                col0 = q * PENW + (j * 28) * NCAND
                pen = qp.tile([H, 28 * NCAND], f32, tag="pen")
                nc.tensor.matmul(
                    pen[:, :ncol],
                    wk[:],
                    bd[:, col0 : col0 + ncol],
                    start=True,
                    stop=True,
                )
                pens.append(pen)

            def enc_blk(b):
                q, j = b // 3, b % 3
                ng = UG[j]
                u0 = q * UBLK + j * 28
                pen = pens[b]
                enc = ep.tile([H, 28 * NCAND], f32, tag="enc")
                nc.vector.tensor_tensor(
                    enc[:, : ng * NCAND].rearrange("v (u k) -> v u k", k=NCAND),
                    lstack[:, :, u0 : u0 + ng].rearrange("v k u -> v u k"),
                    pen[:, : ng * NCAND].rearrange("v (u k) -> v u k", k=NCAND),
                    op=OP.add,
                )
                nc.vector.tensor_reduce(
                    mstar[:, u0 : u0 + ng],
                    enc[:, : ng * NCAND].rearrange("v (u k) -> v u k", k=NCAND),
                    axis=mybir.AxisListType.X,
                    op=OP.min,
                )

            # S PSUM groups span 32 image rows so the DVE eviction lands on
            # a legal 0/32/64 partition boundary.
            G = 32
            sptg = [None]

            def do_chunk(c_, defer_copy=False):
                rows = CHUNK_ROWS[c_]
                ec = xp.tile([C, MAXROWS * W], bf16, tag="ec")
                nc.scalar.activation(
                    ec[:, : rows * W], lcs[c_][:, : rows * W], AF.Exp
                )
                g = GROUP_OF[c_]
                if row0[c_] == g * G:  # first chunk of its group
                    spt_t = sp.tile([G, W], f32, tag="spt")
                    sptg[0] = spt_t
                spt = sptg[0]
                for i in range(rows):
                    m = row0[c_] - g * G + i
                    nc.tensor.matmul(
                        spt[:],
                        slider[:, G - m : 2 * G - m],
                        ec[:, i * W : (i + 1) * W],
                        start=(m == 0),
                        stop=(m == G - 1),
                    )
                if c_ == GROUP_LAST[g] and not defer_copy:
                    do_copy(g)

            def do_copy(g):
                nc.vector.tensor_copy(
                    s_b[g * G : (g + 1) * G, :], sptg[0][:]
                )

            # ---------- focal loss (pixel-major, 32-row groups) ----------
            ln_s = pp.tile([H, W], f32)
            r_i = pp.tile([H, W], i32)
            r_f = pp.tile([H, W], f32)
            lam = pp.tile([H, W], f32)  # lambda* + 16
            logp = pp.tile([H, W], f32)
            p = pp.tile([H, W], f32)
            om = pp.tile([H, W], f32)  # (1 - p)^2
            t1 = pp.tile([H, W], f32)
            wgt = pp.tile([H, W], f32)  # 12 * fg
            wl = pp.tile([H, W], f32)
            part = pp.tile([H, 1], f32)

            # focal runs in two partition-legal row groups: [0:64], [64:96]
            FGRP = (slice(0, 64), slice(64, 96))

            def focalA(g):
                s = FGRP[g]
                nc.scalar.activation(ln_s[s, :], s_b[s, :], AF.Ln)
                # rank extraction: m*/32 - 0.25 lies strictly in (r, r+0.5),
                # so f32->i32 yields r under truncation or rounding alike
                nc.vector.tensor_scalar(
                    r_i[s, :], mstar[s, :], 1.0 / STRIDE, -0.25,
                    op0=OP.mult, op1=OP.add,
                )
                nc.vector.tensor_copy(r_f[s, :], r_i[s, :])
                nc.vector.scalar_tensor_tensor(
                    lam[s, :], r_f[s, :], -STRIDE, mstar[s, :],
                    op0=OP.mult, op1=OP.add,
                )
                nc.vector.scalar_tensor_tensor(
                    logp[s, :], lam[s, :], OFF, ln_s[s, :],
                    op0=OP.subtract, op1=OP.subtract,
                )

            def focalB(g):
                s = FGRP[g]
                nc.scalar.activation(p[s, :], logp[s, :], AF.Exp)
                # (1-p)^2 on DVE: keeps the critical ACT stream shorter
                nc.vector.tensor_scalar(
                    om[s, :], p[s, :], -1.0, 1.0, op0=OP.mult, op1=OP.add
                )
                nc.vector.tensor_tensor(om[s, :], om[s, :], om[s, :], op=OP.mult)
                nc.vector.tensor_tensor(t1[s, :], om[s, :], logp[s, :], op=OP.mult)
                nc.vector.tensor_scalar(
                    wgt[s, :], mstar[s, :], STRIDE * NBOX, 12.0,
                    op0=OP.is_lt, op1=OP.mult,
                )
                nc.vector.scalar_tensor_tensor(
                    wl[s, :], wgt[s, :], 1.0, t1[s, :], op0=OP.add, op1=OP.mult
                )
                nc.vector.tensor_reduce(
                    part[s, :], wl[s, :], axis=mybir.AxisListType.X, op=OP.add
                )

            # Emission order: pen blocks 0-5 run on consts alone; blocks
            # 6-11 (which recycle PSUM banks read by encs 0-5) slot in
            # mid-stream; s_b copies precede the (gather-gated) enc chain
            # on the DVE queue so S PSUM recycling never stalls; focal
            # group work starts as soon as its s_b rows + mstar exist.
            load_chunk(1)
            for b in range(6):
                pen_blk(b)
            do_chunk(0)
            nc.gpsimd.dma_start(
                lstack[:].rearrange("v k u -> v (k u)"), lstack_d[:]
            )
            do_chunk(1)
            load_chunk(3)
            do_chunk(2)
            for b in range(6):
                enc_blk(b)
            for b in range(6, 12):
                pen_blk(b)
            for b in range(6, 12):
                enc_blk(b)
            load_chunk(5)
            do_chunk(3)
            do_chunk(4)
            focalA(0)
            do_chunk(5, defer_copy=True)
            focalB(0)
            do_copy(2)
            focalA(1)
            focalB(1)
            nc.sync.dma_start(out_d[:], part[:])
            if dbg:
                nc.sync.dma_start(dbg_m[:], mstar[:])
                nc.sync.dma_start(dbg_s[:], s_b[:])

    # The ACT-table pass picks the FIRST act_info set containing each
    # function, which makes the focal Ln/Exp interleave reload tables 5x
    # (1.28us each). Blank the shadowing sets (positions preserved, so
    # act_func_set_ids stay consistent with act_info.json) so the combined
    # natural_log_exp_and_others set serves every activation: 1 load total.
    from concourse import hw_specs as _hw

    _orig_tabs = _hw.get_activation_tables
    _shadow = {"exp_and_others", "natural_log", "exp_and_friends"}

    def _patched(arch):
        tabs = _orig_tabs(arch)
        return {k: (set() if k in _shadow else v) for k, v in tabs.items()}

    _hw.get_activation_tables = _patched
    bacc.get_activation_tables = _patched  # bacc binds it via from-import
    try:
        nc.compile()
    finally:
        _hw.get_activation_tables = _orig_tabs
        bacc.get_activation_tables = _orig_tabs
    return nc


def _bin_of(depth):
    """LID bin indices, fp32-exact replica of the reference."""
    d = np.float32(depth)
    bin_size = np.float32(2.0 * (DEPTH_MAX - DEPTH_MIN) / (NUM_BINS * (1 + NUM_BINS)))
    idx = np.float32(-0.5) + np.float32(0.5) * np.sqrt(
        np.float32(1.0) + np.float32(8.0) * (d - np.float32(DEPTH_MIN)) / bin_size
    )
    bad = (idx < 0) | (idx > NUM_BINS) | ~np.isfinite(idx)
    idx = np.where(bad, np.float32(NUM_BINS), idx)
    # the graded reference runs on an XLA build whose f32->s32 convert
    # rounds to nearest, so match that instead of C truncation
    return np.rint(idx).astype(np.int32)


def _host_prep(depth_logits, gt_boxes2d, num_gt_per_img, gt_center_depth):
    """Build the 8 per-core input maps."""
    import ml_dtypes

    n = int(num_gt_per_img)
    boxes = np.asarray(gt_boxes2d, np.float32).reshape(B, n, 4)
    depths = np.asarray(gt_center_depth, np.float32).reshape(B, n)
    logits = np.ascontiguousarray(np.asarray(depth_logits, np.float32).reshape(B, C, F))

    # one-hot column slider for the S matmuls: col 32 is all-ones
    slider = np.zeros((C, 65), np.float32)
    slider[:, 32] = 1.0
    slider = slider.astype(ml_dtypes.bfloat16)

    kk = np.arange(NCAND, dtype=np.float32)
    us = np.arange(W, dtype=np.float32)
    vs = np.arange(H, dtype=np.float32)

    # block "diagonal" rows 0..16 of bd: bd[k', (q, u', k)] = (k == k')
    bd_base = np.zeros((NCAND + 1, 4 * UBLK * NCAND), np.float32)
    for u in range(W):
        bd_base[kk.astype(np.int32), u * NCAND + kk.astype(np.int32)] = 1.0

    in_maps = []
    for i in range(B):
        bins = _bin_of(depths[i])
        order = np.argsort(bins, kind="stable")
        u1 = np.floor(boxes[i, :, 0]).astype(np.float32)[order]
        v1 = np.floor(boxes[i, :, 1]).astype(np.float32)[order]
        u2 = np.ceil(boxes[i, :, 2]).astype(np.float32)[order]
        v2 = np.ceil(boxes[i, :, 3]).astype(np.float32)[order]
        cand = np.concatenate([bins[order], [NUM_BINS]]).astype(np.int32)
        # host-stacked candidate logits: lstack[v, k, u] = L[cand[k], v, u]
        lstack = (
            logits[i][cand]
            .reshape(NCAND, H, W)
            .transpose(1, 0, 2)
            .reshape(H, NCAND * W)
            .astype(ml_dtypes.bfloat16)
        )
        # background slot covers everything
        u1c = np.concatenate([u1, [0.0]]).astype(np.float32)
        u2c = np.concatenate([u2, [W]]).astype(np.float32)
        v1c = np.concatenate([v1, [0.0]]).astype(np.float32)
        v2c = np.concatenate([v2, [H]]).astype(np.float32)

        rowm = ((vs[None, :] >= v1c[:, None]) & (vs[None, :] < v2c[:, None])).astype(
            np.float32
        )  # [17, 96]
        colm = ((us[None, :] >= u1c[:, None]) & (us[None, :] < u2c[:, None])).astype(
            np.float32
        )  # [17, 320]

        wk = np.ones((NCAND + 1, H), np.float32)
        wk[:NCAND] = BIG * (2.0 - rowm)

        bd = bd_base.copy()
        # row 17: colm-dependent encode 32k + 16 - 2048*colm, u-major per quarter
        cvec = (STRIDE * kk[None, :] + OFF) - BIG * colm.T  # [320, 17] (u, k)
        bd[NCAND, :] = cvec.reshape(-1)

        in_maps.append(
            {
                "logitsb": logits[i].astype(ml_dtypes.bfloat16),
                "lstack": lstack,
                "wk": wk.astype(ml_dtypes.bfloat16),
                "bd": bd.astype(ml_dtypes.bfloat16),
                "slider": slider,
            }
        )
    return in_maps


def get_program():
    global _PROG
    if _PROG is None:
        _PROG = _build_program()
    return _PROG


def kernel(depth_logits, gt_boxes2d, num_gt_per_img, gt_center_depth, _trace=False):
    from concourse import bass_utils

    nc = get_program()
    in_maps = _host_prep(depth_logits, gt_boxes2d, num_gt_per_img, gt_center_depth)
    res = bass_utils.run_bass_kernel_spmd(
        nc, in_maps, core_ids=list(range(B)), trace=_trace
    )
    total = np.float64(0.0)
    for r in res.results:
        total += np.float64(r["out"].astype(np.float64).sum())
    loss = np.float32(-ALPHA * total / (B * H * W))
    if _trace:
        kernel._last_results = res
    return np.asarray(loss, dtype=np.float32)


# revision 78
# speedup vs baseline: 1.7611x; 1.7611x over previous
"""DDNLoss (depth-distribution focal loss) Trainium2 kernel, 8-core data-parallel.

Strategy (per core = one image of the batch):
  * depth_logits [81, 30720] loaded in 6 contiguous chunks (issued at t=0),
    ACT exp -> bf16, then 16 one-hot-column matmuls per chunk partition-reduce
    the 81 channels into a [16, 320] PSUM tile whose row i is image row
    16c+i's softmax denominator; one DVE copy evicts it straight into the
    pixel-major s_b [96, 320].
  * The 17 candidate channels (16 sorted box bins + background 80) are
    gathered with 2 indirect DMAs (f32 -> bf16 cast) and restacked to
    lstack [96(v), 17(k), 320(u)] with 17 partition-expand DMAs spread
    across the pool/vector/sync rings.
  * Rasterization (min-depth box wins) is an arithmetic min-encode:
    enc = lambda + 32k + 16 + pen, pen built by ONE K=18 bf16 matmul per
    512-col block: rows 0..16 carry 2048*(2-rowmask[k,v]) through a
    block-diagonal expansion, row 17 broadcasts the column-mask encode
    32k+16-2048*colmask (all values bf16-exact). A strided tensor_reduce
    min over k yields m* = 32k* + 16 + lambda* per pixel.
  * Focal loss is elementwise in pixel-major layout; per-partition row sums
    are returned and the host adds the 8 per-core partials.
"""

import sys

sys.path.insert(0, "/opt/trn_rl_repo")

import numpy as np

B, C, H, W = 8, 81, 96, 320
F = H * W
NBOX, NCAND = 16, 17  # 16 boxes + background
ALPHA = 0.25
FG_W, BG_W = 13.0, 1.0
DEPTH_MIN, DEPTH_MAX, NUM_BINS = 0.001, 60.0, 80

STRIDE = 32.0  # rank stride in the min-encode
OFF = 16.0  # lambda offset so the payload is positive
BIG = 2048.0  # uncovered-box penalty (bf16-exact composites)
UBLK = 80  # u-block size for the pen/enc/reduce pipeline (4 blocks)
# uneven exp/S chunks: small first chunks prime the pipeline, small last
# chunks shorten the tail; groups of 32 rows share one PSUM tile so the
# eviction lands on 0/32/64 partition boundaries
CHUNK_ROWS = (8, 24, 24, 8, 24, 8)
NCHUNK = len(CHUNK_ROWS)
MAXROWS = max(CHUNK_ROWS)


def _groups(chunk_rows):
    group_of, group_last, r = [], [], 0
    for c, n in enumerate(chunk_rows):
        g = r // 32
        group_of.append(g)
        r += n
        assert r <= (g + 1) * 32, "chunk straddles a 32-row psum group"
        if r % 32 == 0:
            group_last.append(c)
    assert r == 96 and len(group_last) == 3
    return tuple(group_of), tuple(group_last)


GROUP_OF, GROUP_LAST = _groups(CHUNK_ROWS)

_PROG = None  # cached program


def _build_program():
    from concourse import bass, bacc, tile, mybir

    f32 = mybir.dt.float32
    bf16 = mybir.dt.bfloat16
    i32 = mybir.dt.int32
    AF = mybir.ActivationFunctionType
    OP = mybir.AluOpType

    nc = bacc.Bacc(
        "TRN2",
        target_bir_lowering=False,
        debug=False,
        enable_asserts=False,
        dynamic_dma_scratch_size=65536,
    )

    # ---- DRAM I/O (per-core) ----
    # bf16 copy of the logits (host-cast): the S path only needs exp() of
    # them and 2^-9 relative error is far inside the loss tolerance, so the
    # kernel streams half the bytes. The f32 original is never read.
    Lb = nc.dram_tensor("logitsb", [C, F], bf16, kind="ExternalInput")
    lstack_d = nc.dram_tensor("lstack", [H, NCAND * W], bf16, kind="ExternalInput")
    wk_d = nc.dram_tensor("wk", [NCAND + 1, H], bf16, kind="ExternalInput")
    bd_d = nc.dram_tensor("bd", [NCAND + 1, 4 * UBLK * NCAND], bf16, kind="ExternalInput")
    slider_d = nc.dram_tensor("slider", [C, 65], bf16, kind="ExternalInput")
    out_d = nc.dram_tensor("out", [H, 1], f32, kind="ExternalOutput")
    import os

    dbg = os.environ.get("KERNEL_DEBUG") == "1"
    if dbg:
        dbg_m = nc.dram_tensor("dbg_m", [H, W], f32, kind="ExternalOutput")
        dbg_s = nc.dram_tensor("dbg_s", [H, W], f32, kind="ExternalOutput")

    PENW = UBLK * NCAND  # 1360 pen columns per quarter

    with tile.TileContext(nc) as tc:
        with (
            tc.tile_pool(name="persist", bufs=1) as pp,
            tc.tile_pool(name="lchunk", bufs=6) as cp,
            tc.tile_pool(name="echunk", bufs=2) as xp,
            tc.tile_pool(name="enc", bufs=1) as ep,
            tc.tile_pool(name="spsum", bufs=2, space="PSUM") as sp,
            tc.tile_pool(name="ppsum", bufs=6, space="PSUM") as qp,
        ):
            # ---------- constant / small input loads first, then chunk 0,
            # then the host-stacked candidate tensor (1MB, gates the enc
            # chain), then the remaining logits chunks. All plain sync-ring
            # HWDGE loads: no SWDGE, no gpsimd, no drains.
            row0 = np.cumsum((0,) + CHUNK_ROWS)
            slider = pp.tile([C, 65], bf16)
            wk = pp.tile([NCAND + 1, H], bf16)
            bd = pp.tile([NCAND + 1, 4 * PENW], bf16)

            def load_consts():
                nc.sync.dma_start(slider[:], slider_d[:])
                nc.sync.dma_start(wk[:], wk_d[:])
                nc.sync.dma_start(bd[:], bd_d[:])

            # Loads alternate HWDGE (sync, queues 0-8) and SWDGE (gpsimd,
            # queues 9-15) rings so all 16 DMA queues stream. Sync-ring
            # completions are tracked per-DMA, so those all issue upfront;
            # SWDGE completions are tracked by a coarse FIFO tick (any
            # compute emitted after a pool DMA waits for ALL earlier pool
            # DMAs), so the pool-ring dma_starts are interleaved into the
            # compute emission just before their consumers.
            lcs = {}

            def load_chunk(c_):
                lc = cp.tile([C, MAXROWS * W], bf16, tag="lc")
                ring = nc.sync if c_ % 2 == 0 else nc.gpsimd
                ring.dma_start(
                    lc[:, : CHUNK_ROWS[c_] * W],
                    Lb[:, row0[c_] * W : row0[c_ + 1] * W],
                )
                lcs[c_] = lc

            load_consts()
            load_chunk(0)
            load_chunk(2)
            load_chunk(4)
            lstack = pp.tile([H, NCAND, W], bf16)

            # ---------- pen matmuls (bf16, K=18) ----------
            # pen[v, u*17+k] = 2048*(2-rowm[k,v]) + 32k+16 - 2048*colm[k,u]
            # Twelve 1-PSUM-bank blocks of 28/28/24 u-groups per quarter so
            # pen matmuls never couple the PE queue to the (gather-gated)
            # enc chain: with bufs=6, block b+6 only recycles a bank whose
            # enc read finished long before.
            pens = []
            s_b = pp.tile([H, W], f32)  # softmax denominator, pixel-major
            mstar = pp.tile([H, W], f32)
            UG = (28, 28, 24)  # u-groups per block (x17 cols)

            def pen_blk(b):
                q, j = b // 3, b % 3
                ncol = UG[j] * NCAND
                col0 = q * PENW + (j * 28) * NCAND
                pen = qp.tile([H, 28 * NCAND], f32, tag="pen")
                nc.tensor.matmul(
                    pen[:, :ncol],
                    wk[:],
                    bd[:, col0 : col0 + ncol],
                    start=True,
                    stop=True,
                )
                pens.append(pen)

            def enc_blk(b):
                q, j = b // 3, b % 3
                ng = UG[j]
                u0 = q * UBLK + j * 28
                pen = pens[b]
                enc = ep.tile([H, 28 * NCAND], f32, tag="enc")
                nc.vector.tensor_tensor(
                    enc[:, : ng * NCAND].rearrange("v (u k) -> v u k", k=NCAND),
                    lstack[:, :, u0 : u0 + ng].rearrange("v k u -> v u k"),
                    pen[:, : ng * NCAND].rearrange("v (u k) -> v u k", k=NCAND),
                    op=OP.add,
                )
                nc.vector.tensor_reduce(
                    mstar[:, u0 : u0 + ng],
                    enc[:, : ng * NCAND].rearrange("v (u k) -> v u k", k=NCAND),
                    axis=mybir.AxisListType.X,
                    op=OP.min,
                )

            # S PSUM groups span 32 image rows so the DVE eviction lands on
            # a legal 0/32/64 partition boundary.
            G = 32
            sptg = [None]

            def do_chunk(c_, defer_copy=False):
                rows = CHUNK_ROWS[c_]
                ec = xp.tile([C, MAXROWS * W], bf16, tag="ec")
                nc.scalar.activation(
                    ec[:, : rows * W], lcs[c_][:, : rows * W], AF.Exp
                )
                g = GROUP_OF[c_]
                if row0[c_] == g * G:  # first chunk of its group
                    spt_t = sp.tile([G, W], f32, tag="spt")
                    sptg[0] = spt_t
                spt = sptg[0]
                for i in range(rows):
                    m = row0[c_] - g * G + i
                    nc.tensor.matmul(
                        spt[:],
                        slider[:, G - m : 2 * G - m],
                        ec[:, i * W : (i + 1) * W],
                        start=(m == 0),
                        stop=(m == G - 1),
                    )
                if c_ == GROUP_LAST[g] and not defer_copy:
                    do_copy(g)

            def do_copy(g):
                nc.vector.tensor_copy(
                    s_b[g * G : (g + 1) * G, :], sptg[0][:]
                )

            # ---------- focal loss (pixel-major, 32-row groups) ----------
            ln_s = pp.tile([H, W], f32)
            r_i = pp.tile([H, W], i32)
            r_f = pp.tile([H, W], f32)
            lam = pp.tile([H, W], f32)  # lambda* + 16
            logp = pp.tile([H, W], f32)
            p = pp.tile([H, W], f32)
            om = pp.tile([H, W], f32)  # (1 - p)^2
            t1 = pp.tile([H, W], f32)
            wgt = pp.tile([H, W], f32)  # 12 * fg
            wl = pp.tile([H, W], f32)
            part = pp.tile([H, 1], f32)

            # focal runs in two partition-legal row groups: [0:64], [64:96]
            FGRP = (slice(0, 64), slice(64, 96))

            def focalA(g):
                s = FGRP[g]
                nc.scalar.activation(ln_s[s, :], s_b[s, :], AF.Ln)
                # rank extraction: m*/32 - 0.25 lies strictly in (r, r+0.5),
                # so f32->i32 yields r under truncation or rounding alike
                nc.vector.tensor_scalar(
                    r_i[s, :], mstar[s, :], 1.0 / STRIDE, -0.25,
                    op0=OP.mult, op1=OP.add,
                )
                nc.vector.tensor_copy(r_f[s, :], r_i[s, :])
                nc.vector.scalar_tensor_tensor(
                    lam[s, :], r_f[s, :], -STRIDE, mstar[s, :],
                    op0=OP.mult, op1=OP.add,
                )
                nc.vector.scalar_tensor_tensor(
                    logp[s, :], lam[s, :], OFF, ln_s[s, :],
                    op0=OP.subtract, op1=OP.subtract,
                )

            def focalB(g):
                s = FGRP[g]
                nc.scalar.activation(p[s, :], logp[s, :], AF.Exp)
                # (1-p)^2 on DVE: keeps the critical ACT stream shorter
                nc.vector.tensor_scalar(
                    om[s, :], p[s, :], -1.0, 1.0, op0=OP.mult, op1=OP.add
                )
                nc.vector.tensor_tensor(om[s, :], om[s, :], om[s, :], op=OP.mult)
                nc.vector.tensor_tensor(t1[s, :], om[s, :], logp[s, :], op=OP.mult)
                nc.vector.tensor_scalar(
                    wgt[s, :], mstar[s, :], STRIDE * NBOX, 12.0,
                    op0=OP.is_lt, op1=OP.mult,
                )
                nc.vector.scalar_tensor_tensor(
                    wl[s, :], wgt[s, :], 1.0, t1[s, :], op0=OP.add, op1=OP.mult
                )
                nc.vector.tensor_reduce(
                    part[s, :], wl[s, :], axis=mybir.AxisListType.X, op=OP.add
                )

            # Emission order: pen blocks 0-5 run on consts alone; blocks
            # 6-11 (which recycle PSUM banks read by encs 0-5) slot in
            # mid-stream; s_b copies precede the (gather-gated) enc chain
            # on the DVE queue so S PSUM recycling never stalls; focal
            # group work starts as soon as its s_b rows + mstar exist.
            for b in range(6):
                pen_blk(b)
            do_chunk(0)
            load_chunk(1)
            do_chunk(1)
            nc.gpsimd.dma_start(
                lstack[:].rearrange("v k u -> v (k u)"), lstack_d[:]
            )
            load_chunk(3)
            do_chunk(2)
            for b in range(6):
                enc_blk(b)
            for b in range(6, 12):
                pen_blk(b)
            for b in range(6, 12):
                enc_blk(b)
            load_chunk(5)
            do_chunk(3)
            do_chunk(4)
            focalA(0)
            do_chunk(5, defer_copy=True)
            focalB(0)
            do_copy(2)
            focalA(1)
            focalB(1)
            nc.sync.dma_start(out_d[:], part[:])
            if dbg:
                nc.sync.dma_start(dbg_m[:], mstar[:])
                nc.sync.dma_start(dbg_s[:], s_b[:])

    # The ACT-table pass picks the FIRST act_info set containing each
    # function, which makes the focal Ln/Exp interleave reload tables 5x
    # (1.28us each). Blank the shadowing sets (positions preserved, so
    # act_func_set_ids stay consistent with act_info.json) so the combined
    # natural_log_exp_and_others set serves every activation: 1 load total.
    from concourse import hw_specs as _hw

    _orig_tabs = _hw.get_activation_tables
    _shadow = {"exp_and_others", "natural_log", "exp_and_friends"}

    def _patched(arch):
        tabs = _orig_tabs(arch)
        return {k: (set() if k in _shadow else v) for k, v in tabs.items()}

    _hw.get_activation_tables = _patched
    bacc.get_activation_tables = _patched  # bacc binds it via from-import
    try:
        nc.compile()
    finally:
        _hw.get_activation_tables = _orig_tabs
        bacc.get_activation_tables = _orig_tabs
    return nc


def _bin_of(depth):
    """LID bin indices, fp32-exact replica of the reference."""
    d = np.float32(depth)
    bin_size = np.float32(2.0 * (DEPTH_MAX - DEPTH_MIN) / (NUM_BINS * (1 + NUM_BINS)))
    idx = np.float32(-0.5) + np.float32(0.5) * np.sqrt(
        np.float32(1.0) + np.float32(8.0) * (d - np.float32(DEPTH_MIN)) / bin_size
    )
    bad = (idx < 0) | (idx > NUM_BINS) | ~np.isfinite(idx)
    idx = np.where(bad, np.float32(NUM_BINS), idx)
    # the graded reference runs on an XLA build whose f32->s32 convert
    # rounds to nearest, so match that instead of C truncation
    return np.rint(idx).astype(np.int32)


def _host_prep(depth_logits, gt_boxes2d, num_gt_per_img, gt_center_depth):
    """Build the 8 per-core input maps."""
    import ml_dtypes

    n = int(num_gt_per_img)
    boxes = np.asarray(gt_boxes2d, np.float32).reshape(B, n, 4)
    depths = np.asarray(gt_center_depth, np.float32).reshape(B, n)
    logits = np.ascontiguousarray(np.asarray(depth_logits, np.float32).reshape(B, C, F))

    # one-hot column slider for the S matmuls: col 32 is all-ones
    slider = np.zeros((C, 65), np.float32)
    slider[:, 32] = 1.0
    slider = slider.astype(ml_dtypes.bfloat16)

    kk = np.arange(NCAND, dtype=np.float32)
    us = np.arange(W, dtype=np.float32)
    vs = np.arange(H, dtype=np.float32)

    # block "diagonal" rows 0..16 of bd: bd[k', (q, u', k)] = (k == k')
    bd_base = np.zeros((NCAND + 1, 4 * UBLK * NCAND), np.float32)
    for u in range(W):
        bd_base[kk.astype(np.int32), u * NCAND + kk.astype(np.int32)] = 1.0

    in_maps = []
    for i in range(B):
        bins = _bin_of(depths[i])
        order = np.argsort(bins, kind="stable")
        u1 = np.floor(boxes[i, :, 0]).astype(np.float32)[order]
        v1 = np.floor(boxes[i, :, 1]).astype(np.float32)[order]
        u2 = np.ceil(boxes[i, :, 2]).astype(np.float32)[order]
        v2 = np.ceil(boxes[i, :, 3]).astype(np.float32)[order]
        cand = np.concatenate([bins[order], [NUM_BINS]]).astype(np.int32)
        # host-stacked candidate logits: lstack[v, k, u] = L[cand[k], v, u]
        lstack = (
            logits[i][cand]
            .reshape(NCAND, H, W)
            .transpose(1, 0, 2)
            .reshape(H, NCAND * W)
            .astype(ml_dtypes.bfloat16)
        )
        # background slot covers everything
        u1c = np.concatenate([u1, [0.0]]).astype(np.float32)
        u2c = np.concatenate([u2, [W]]).astype(np.float32)
        v1c = np.concatenate([v1, [0.0]]).astype(np.float32)
        v2c = np.concatenate([v2, [H]]).astype(np.float32)

        rowm = ((vs[None, :] >= v1c[:, None]) & (vs[None, :] < v2c[:, None])).astype(
            np.float32
        )  # [17, 96]
        colm = ((us[None, :] >= u1c[:, None]) & (us[None, :] < u2c[:, None])).astype(
            np.float32
        )  # [17, 320]

        wk = np.ones((NCAND + 1, H), np.float32)
        wk[:NCAND] = BIG * (2.0 - rowm)

        bd = bd_base.copy()
        # row 17: colm-dependent encode 32k + 16 - 2048*colm, u-major per quarter
        cvec = (STRIDE * kk[None, :] + OFF) - BIG * colm.T  # [320, 17] (u, k)
        bd[NCAND, :] = cvec.reshape(-1)

        in_maps.append(
            {
                "logitsb": logits[i].astype(ml_dtypes.bfloat16),
                "lstack": lstack,
                "wk": wk.astype(ml_dtypes.bfloat16),
                "bd": bd.astype(ml_dtypes.bfloat16),
                "slider": slider,
            }
        )
    return in_maps


def get_program():
    global _PROG
    if _PROG is None:
        _PROG = _build_program()
    return _PROG


def kernel(depth_logits, gt_boxes2d, num_gt_per_img, gt_center_depth, _trace=False):
    from concourse import bass_utils

    nc = get_program()
    in_maps = _host_prep(depth_logits, gt_boxes2d, num_gt_per_img, gt_center_depth)
    res = bass_utils.run_bass_kernel_spmd(
        nc, in_maps, core_ids=list(range(B)), trace=_trace
    )
    total = np.float64(0.0)
    for r in res.results:
        total += np.float64(r["out"].astype(np.float64).sum())
    loss = np.float32(-ALPHA * total / (B * H * W))
    if _trace:
        kernel._last_results = res
    return np.asarray(loss, dtype=np.float32)
